# revision 23
# baseline (speedup 1.0000x reference)
"""Trainium2 Bass kernel for the FFT-stacked hyperbolic-BN MLP block.

Math notes (why the device kernel is so simple):

  reference: h  = relu(BN(x@W1 + b1))
             u  = logmap_c(h)          (Poincare ball, c=0.001)
             v  = Re(ifft(fft(u) * H_eff)),  H_eff = exp(L*log(g_real + i g_imag))
             y  = expmap_c(v)
             h3 = relu(BN(alpha*y + beta_p*h))
             out= h3@W2 + b2

  * b1 cancels inside batchnorm (mean subtraction), so it is dropped.
  * With H_eff == 1 (the case whenever g_real==1, g_imag==0, since
    exp(L*log(1)) == 1 exactly in complex fp32), the fft chain is the
    identity:  v == u, and expmap(logmap(h)) collapses to
       y = h * min(1, (1-1e-5)/(sc*|h|)),  so
       alpha*y + beta_p*h = (alpha*min(1,R/|h|) + beta_p) * h =: g(row) * h.
  * More generally the fft chain is a circulant convolution with the real
    kernel Re(ifft(H_eff)); we check at run time that this kernel is a delta
    (it is, for the shipped inputs) and otherwise fall back to a faithful
    numpy implementation of the whole reference.

Device pipeline per core (batch-sharded, 1024 rows/core, 8 cores):

  P1 (three fp8e4m3 DoubleRow passes, residual-corrected):
     z*128 = x(8)Wa + (32(x-x8))(8)(Wa/32) + (x/16)(8)(16(W*128-Wa))
     Terms are host-quantized so every PSUM accumulation carries the same
     2^7 scale; passes run small-to-large (c, b, a) so the bf16 z
     accumulator never rounds a large running value against a small term.
     Pass a's PSUM->SBUF evacuation on DVE carries accum_out (BN1 column
     sums); an ACT Square pass accumulates sum(z^2). DoubleRow processes
     2 k-tiles/instruction at 0.5 cycles/row: 4x the bf16 matmul rate.
  BN1: stats exchanged per column chunk (10|10|8|4 ht tiles) through a
     DRAM AllGather (cheaper than AllReduce in latency and off the
     critical path for all but the last chunk), summed locally on DVE,
     then h = relu(scale*z+bias) in place (bf16).
  Norms: DVE squares + PE ones-matmul partition reduction, interleaved
     into the P1 instruction stream chunk by chunk.
  g row-scales: computed on one partition, broadcast to 128 partitions
     with a rank-1 f32 matmul on PE (no DRAM round trip).
  P2: h2 = g*h on DVE (4x bf16 mode) with fused BN2 column sums;
     sum(h2^2) split across ACT/DVE. BN2 stats exchanged in 3 chunks
     (8|12|12) so P3 can start while later chunks are still in flight.
  P3: out = h3 @ W2 + b2 (bf16, h3 slices stationary), two bt passes of
     8 PSUM banks each; W2 streamed bf16; bias added on DVE during
     evacuation; pass-A rows DMA out while pass B computes.
"""

import os
import sys

sys.path.insert(0, "/opt/trn_rl_repo")

import numpy as np
import ml_dtypes

F8NP = ml_dtypes.float8_e4m3
BF16NP = ml_dtypes.bfloat16

B_FULL = 8192
D_IN = 3072
D_H = 4096
D_OUT = 1000
N_CORES = 8
B_SH = B_FULL // N_CORES          # 1024 rows per core
KT = D_IN // 128                  # 24 k-tiles
KP = KT // 2                      # 12 DoubleRow k-pairs
HT = D_H // 128                   # 32 h-tiles
HG = HT // 2                      # 16 groups of 2 ht (256 cols)
BT = B_SH // 128                  # 8 row-tiles per core

C_CURV = 0.001
EPS = 1e-7
BN_EPS = 1e-5
L_EXP = 100000000
SC = float(np.sqrt(np.float32(C_CURV)))
R_CLIP = float((1.0 - 1e-5) / SC)   # radius above which rows get rescaled

S_W = 128.0                       # power-of-2 scale on W1 (fp8 subnormal guard)
S_XB = 32.0                       # scale on the x residual term
S_WC = 16.0                       # scale on the W residual term

BN1_CHUNKS = [10, 10, 8, 4]       # ht tiles per BN1 stats exchange
BN2_CHUNKS = [6, 12, 14]          # ht tiles per BN2 stats exchange

_BUILD_CACHE = {}


def _filter_kernel(g_real, g_imag):
    """Real circulant kernel of the fft->*H_eff->ifft chain (complex64 math,
    mirroring the reference)."""
    H = g_real.astype(np.complex64) + 1j * g_imag.astype(np.complex64)
    H_eff = np.exp(np.complex64(L_EXP) * np.log(H))
    return np.fft.ifft(H_eff)


def _np_reference(x, W1, b1, gamma1, beta1, g_real, g_imag, alpha, beta_p,
                  gamma2, beta2, W2, b2):
    """Faithful numpy fallback for non-delta spectral filters."""
    def bn(a, gamma, beta):
        mu = a.mean(0)
        var = a.var(0)
        return gamma * (a - mu) / np.sqrt(var + BN_EPS) + beta

    def logmap(h):
        n = np.linalg.norm(h, axis=1, keepdims=True)
        scn = np.clip(SC * n, EPS, 1.0 - 1e-5)
        return np.arctanh(scn) * h / np.maximum(SC * n, EPS)

    def expmap(v):
        n = np.maximum(np.linalg.norm(v, axis=1, keepdims=True), EPS)
        return np.tanh(SC * n) * v / (SC * n)

    h = np.maximum(bn(x @ W1 + b1, gamma1, beta1), 0.0)
    u = logmap(h)
    U = np.fft.fft(u, axis=1)
    H = g_real.astype(np.complex64) + 1j * g_imag.astype(np.complex64)
    H_eff = np.exp(np.complex64(L_EXP) * np.log(H))
    v = np.real(np.fft.ifft(U * H_eff[None, :], axis=1)).astype(np.float32)
    y = expmap(v)
    h2 = alpha * y + beta_p * h
    h3 = np.maximum(bn(h2, gamma2, beta2), 0.0)
    return (h3 @ W2 + b2).astype(np.float32)


def _build():
    import concourse.bacc as bacc
    import concourse.mybir as mybir
    import concourse.tile as tile

    f32 = mybir.dt.float32
    bf16 = mybir.dt.bfloat16
    fp8 = mybir.dt.float8e4
    AFT = mybir.ActivationFunctionType
    ALU = mybir.AluOpType
    DR = mybir.MatmulPerfMode.DoubleRow
    GROUPS = [list(range(N_CORES))]

    nc = bacc.Bacc("TRN2", target_bir_lowering=False, debug=False,
                   num_devices=N_CORES)

    # per-core fp8 term operands, packed [128, kp, j, b]: k = kp*256+j*128+p
    x_in = [nc.dram_tensor(f"x_{t}", [128, KP, 2, B_SH], fp8,
                           kind="ExternalInput") for t in "abc"]
    # shared W1 terms, packed [128, hg, term, kp, j, c(256)]
    W1p = nc.dram_tensor("W1p", [128, HG, 3, KP, 2, 256], fp8,
                         kind="ExternalInput")
    # [4096] -> [128, 32] partition-major
    gamma1 = nc.dram_tensor("gamma1", [128, HT], f32, kind="ExternalInput")
    beta1 = nc.dram_tensor("beta1", [128, HT], f32, kind="ExternalInput")
    gamma2 = nc.dram_tensor("gamma2", [128, HT], f32, kind="ExternalInput")
    beta2 = nc.dram_tensor("beta2", [128, HT], f32, kind="ExternalInput")
    alpha_e = nc.dram_tensor("alpha", [1], f32, kind="ExternalInput")
    beta_p_e = nc.dram_tensor("beta_p", [1], f32, kind="ExternalInput")
    W2p = nc.dram_tensor("W2p", [128, HT, D_OUT], bf16, kind="ExternalInput")
    b2 = nc.dram_tensor("b2", [D_OUT], f32, kind="ExternalInput")
    out = nc.dram_tensor("out", [B_SH, D_OUT], f32, kind="ExternalOutput")

    # BN stats exchange buffers (DRAM AllGather staging)
    cc1_ins = [nc.dram_tensor(f"cc1_in{q}", [128, ch, 4], f32)
               for q, ch in enumerate(BN1_CHUNKS)]
    cc1_outs = [nc.dram_tensor(f"cc1_out{q}", [N_CORES, 128, ch, 4], f32,
                               addr_space="Shared")
                for q, ch in enumerate(BN1_CHUNKS)]
    cc2_ins = [nc.dram_tensor(f"cc2_in{q}", [128, ch, 2], f32)
               for q, ch in enumerate(BN2_CHUNKS)]
    cc2_outs = [nc.dram_tensor(f"cc2_out{q}", [N_CORES, 128, ch, 2], f32,
                               addr_space="Shared")
                for q, ch in enumerate(BN2_CHUNKS)]

    bn1_first = np.cumsum([0] + BN1_CHUNKS)[:-1]
    bn2_first = np.cumsum([0] + BN2_CHUNKS)[:-1]

    def bn1_chunk_of(ht):
        for q, f in enumerate(bn1_first):
            if f <= ht < f + BN1_CHUNKS[q]:
                return q, ht - f
        raise AssertionError

    def bn2_chunk_of(ht):
        for q, f in enumerate(bn2_first):
            if f <= ht < f + BN2_CHUNKS[q]:
                return q, ht - f
        raise AssertionError

    with tile.TileContext(nc) as tc:
        with tc.tile_pool(name="consts", bufs=1) as consts:
            g1 = consts.tile([128, HT], f32)
            bt1 = consts.tile([128, HT], f32)
            g2 = consts.tile([128, HT], f32)
            bt2 = consts.tile([128, HT], f32)
            nc.sync.dma_start(out=g1[:], in_=gamma1[:])
            nc.sync.dma_start(out=bt1[:], in_=beta1[:])
            nc.sync.dma_start(out=g2[:], in_=gamma2[:])
            nc.sync.dma_start(out=bt2[:], in_=beta2[:])
            b2b = consts.tile([128, D_OUT], f32)
            nc.sync.dma_start(out=b2b[:], in_=b2[None, :].to_broadcast([128, D_OUT]))
            ab_sb = consts.tile([1, 2], f32)
            nc.sync.dma_start(out=ab_sb[0:1, 0:1], in_=alpha_e[None, :])
            nc.sync.dma_start(out=ab_sb[0:1, 1:2], in_=beta_p_e[None, :])
            ones_f32 = consts.tile([128, 1], f32)
            nc.vector.memset(ones_f32[:], 1.0)
            ones_bf = consts.tile([128, 1], bf16)
            nc.scalar.activation(ones_bf[:], ones_f32[:], AFT.Identity)
            ones_row = consts.tile([1, 128], f32)
            nc.vector.memset(ones_row[:], 1.0)
            eps_col = consts.tile([128, 1], f32)
            nc.vector.memset(eps_col[:], BN_EPS)

            st1 = [consts.tile([128, ch, 4], f32, name=f"st1_{q}")
                   for q, ch in enumerate(BN1_CHUNKS)]
            st2 = [consts.tile([128, ch, 2], f32, name=f"st2_{q}")
                   for q, ch in enumerate(BN2_CHUNKS)]
            ag1 = [consts.tile([128, N_CORES, ch, 4], f32, name=f"ag1_{q}")
                   for q, ch in enumerate(BN1_CHUNKS)]
            ag2 = [consts.tile([128, N_CORES, ch, 2], f32, name=f"ag2_{q}")
                   for q, ch in enumerate(BN2_CHUNKS)]
            scale1 = consts.tile([128, HT], f32)
            bias1 = consts.tile([128, HT], f32)
            scale2 = consts.tile([128, HT], f32)
            bias2 = consts.tile([128, HT], f32)
            tmps = [consts.tile([128, 16], f32, name=f"tmp{i}")
                    for i in range(3)]
            gvec = consts.tile([1, B_SH], f32)
            gb_sb = consts.tile([128, B_SH], bf16)

            def ag_reduce(ag_t, tot_ap):
                """Sum [128, 8, ...] over ranks into tot (3-level tree)."""
                nc.vector.tensor_add(ag_t[:, 0:4], ag_t[:, 0:4], ag_t[:, 4:8])
                nc.vector.tensor_add(ag_t[:, 0:2], ag_t[:, 0:2], ag_t[:, 2:4])
                nc.vector.tensor_add(tot_ap, ag_t[:, 0], ag_t[:, 1])

            def bn_coeffs(sums, sqs, scl, bia, gbase, bbase, off, ch, t0, t1, t2):
                # mu = sums/B ; var = sqs/B - mu^2
                nc.vector.tensor_scalar_mul(t0[:, 0:ch], sums, 1.0 / B_FULL)
                nc.vector.tensor_scalar_mul(t1[:, 0:ch], sqs, 1.0 / B_FULL)
                nc.vector.tensor_mul(t2[:, 0:ch], t0[:, 0:ch], t0[:, 0:ch])
                nc.vector.tensor_sub(t1[:, 0:ch], t1[:, 0:ch], t2[:, 0:ch])
                nc.scalar.activation(t1[:, 0:ch], t1[:, 0:ch], AFT.Sqrt,
                                     bias=eps_col[:])
                nc.vector.reciprocal(t1[:, 0:ch], t1[:, 0:ch])
                nc.vector.tensor_mul(scl[:, off:off + ch],
                                     gbase[:, off:off + ch], t1[:, 0:ch])
                nc.vector.tensor_mul(t2[:, 0:ch], t0[:, 0:ch],
                                     scl[:, off:off + ch])
                nc.vector.tensor_sub(bia[:, off:off + ch],
                                     bbase[:, off:off + ch], t2[:, 0:ch])

            # ---------------- P1: z = x@W1/S in 3 fp8-DR passes -------------
            zp = tc.tile_pool(name="z", bufs=1)
            z_cm = zp.__enter__()
            z_sb = z_cm.tile([128, HT, B_SH], bf16)

            sqp_cm = tc.tile_pool(name="sq", bufs=3)
            sqp = sqp_cm.__enter__()

            # W2 preload (first output-column half during P1; rest later)
            w2sp_cm = tc.tile_pool(name="w2s", bufs=1)
            w2sp = w2sp_cm.__enter__()
            w2_sb = [w2sp.tile([128, HT, 512], bf16, name="w2h0"), None]

            with tc.tile_pool(name="xt", bufs=2) as xtp, \
                 tc.tile_pool(name="w1", bufs=3) as w1p, \
                 tc.tile_pool(name="ps1", bufs=4, space="PSUM") as pp1, \
                 tc.tile_pool(name="psn", bufs=1, space="PSUM") as ppn:
                n2ps = [ppn.tile([1, 512], f32, tag=f"n2_{i}", name=f"n2_{i}")
                        for i in range(2)]

                xts = {}
                xts["c"] = xtp.tile([128, KP, 2, B_SH], fp8, tag="x", name="xt_c")
                nc.sync.dma_start(out=xts["c"][:], in_=x_in[2][:])

                def load_w1(term, hg):
                    w1t = w1p.tile([128, KP, 2, 256], fp8, tag="w1t")
                    nc.scalar.dma_start(out=w1t[:], in_=W1p[:, hg, term])
                    return w1t

                pending_norm = []  # ht tiles relu'd but norm work deferred

                def emit_norms(upto_len, eng):
                    # squares + partition-reduce matmuls, deferred so the
                    # in-order PE stream trails each chunk's AllGather
                    while len(pending_norm) > upto_len:
                        ht = pending_norm.pop(0)
                        sq = sqp.tile([128, B_SH], bf16, tag="sqn")
                        eng.tensor_mul(sq[:], z_sb[:, ht, :], z_sb[:, ht, :])
                        for bc in range(2):
                            nc.tensor.matmul(
                                n2ps[bc][:], ones_bf[:],
                                sq[:, bc * 512:(bc + 1) * 512],
                                start=(ht == 0), stop=(ht == HT - 1))

                def bn1_finish_chunk(q):
                    """AllGather chunk q stats, compute coeffs, relu+square."""
                    ch = BN1_CHUNKS[q]
                    f = int(bn1_first[q])
                    nc.sync.dma_start(out=cc1_ins[q][:], in_=st1[q][:])
                    nc.gpsimd.collective_compute(
                        "AllGather", mybir.AluOpType.bypass,
                        replica_groups=GROUPS,
                        ins=[cc1_ins[q][:]], outs=[cc1_outs[q][:]])
                    nc.sync.dma_start(
                        out=ag1[q][:],
                        in_=cc1_outs[q].rearrange("r p c x -> p r c x"))
                    ag_reduce(ag1[q], st1[q][:])
                    sums = tmps[0]
                    sqs = tmps[1]
                    nc.vector.tensor_add(sums[:, 0:ch], st1[q][:, :, 0],
                                         st1[q][:, :, 1])
                    nc.vector.tensor_add(sqs[:, 0:ch], st1[q][:, :, 2],
                                         st1[q][:, :, 3])
                    bn_coeffs(sums[:, 0:ch], sqs[:, 0:ch], scale1, bias1,
                              g1, bt1, f, ch, tmps[2], sqs, sums)
                    # relu in place; squares + norm matmuls are deferred
                    for i in range(ch):
                        ht = f + i
                        nc.scalar.activation(
                            z_sb[:, ht, :], z_sb[:, ht, :], AFT.Relu,
                            bias=bias1[:, ht:ht + 1],
                            scale=scale1[:, ht:ht + 1])
                        pending_norm.append(ht)

                for pi, t in enumerate("cba"):
                    term = {"a": 0, "b": 1, "c": 2}[t]
                    w1_next = load_w1(term, 0)
                    if t == "c":
                        xts["b"] = xtp.tile([128, KP, 2, B_SH], fp8, tag="x", name="xt_b")
                        nc.sync.dma_start(out=xts["b"][:], in_=x_in[1][:])
                    if t == "b":
                        xts["a"] = xtp.tile([128, KP, 2, B_SH], fp8, tag="x", name="xt_a")
                        nc.sync.dma_start(out=xts["a"][:], in_=x_in[0][:])
                    for hg in range(HG):
                        w1t = w1_next
                        if hg + 1 < HG:
                            w1_next = load_w1(term, hg + 1)
                        elif pi < 2:
                            w1_next = load_w1({"c": 1, "b": 0}[t], 0)
                        if t == "a" and hg % 4 == 1:
                            # stream first W2 half in during the last pass
                            q4 = (hg - 1) // 4
                            nc.sync.dma_start(
                                out=w2_sb[0][:, q4 * 8:(q4 + 1) * 8],
                                in_=W2p[:, q4 * 8:(q4 + 1) * 8, 0:512])
                        for hh in range(2):
                            ht = hg * 2 + hh
                            if t == "a":
                                emit_norms(7, nc.gpsimd)
                            for bc in range(2):
                                ps = pp1.tile([128, 512], f32, tag="ps")
                                for kp in range(KP):
                                    nc.tensor.matmul(
                                        ps[:],
                                        w1t[:, kp, :, hh * 128:(hh + 1) * 128],
                                        xts[t][:, kp, :,
                                               bc * 512:(bc + 1) * 512],
                                        start=(kp == 0), stop=(kp == KP - 1),
                                        perf_mode=DR)
                                if t == "c":
                                    nc.scalar.activation(
                                        z_sb[:, ht, bc * 512:(bc + 1) * 512],
                                        ps[:], AFT.Copy, scale=1.0 / S_W)
                                else:
                                    q, i = bn1_chunk_of(ht)
                                    acc = (dict(accum_out=st1[q][:, i, bc:bc + 1])
                                           if t == "a" else {})
                                    nc.vector.scalar_tensor_tensor(
                                        out=z_sb[:, ht, bc * 512:(bc + 1) * 512],
                                        in0=ps[:], scalar=1.0 / S_W,
                                        in1=z_sb[:, ht, bc * 512:(bc + 1) * 512],
                                        op0=ALU.mult, op1=ALU.add, **acc)
                                    if t == "a":
                                        sq = sqp.tile([128, 512], bf16,
                                                      tag="sq1")
                                        nc.scalar.activation(
                                            sq[:],
                                            z_sb[:, ht, bc * 512:(bc + 1) * 512],
                                            AFT.Square,
                                            accum_out=st1[q][:, i, 2 + bc:3 + bc])
                            if t == "a":
                                q, i = bn1_chunk_of(ht)
                                if i == BN1_CHUNKS[q] - 1:
                                    bn1_finish_chunk(q)
                emit_norms(0, nc.vector)

                # ---- g row-scales: g = alpha*min(1, R/|h|) + beta_p
                nc.vector.tensor_copy(gvec[0:1, 0:512], n2ps[0][:])
                nc.vector.tensor_copy(gvec[0:1, 512:1024], n2ps[1][:])

            nc.scalar.activation(gvec[0:1, :], gvec[0:1, :], AFT.Sqrt)
            nc.vector.reciprocal(gvec[0:1, :], gvec[0:1, :])
            nc.vector.tensor_scalar(
                out=gvec[0:1, :], in0=gvec[0:1, :],
                scalar1=R_CLIP, scalar2=1.0, op0=ALU.mult, op1=ALU.min)
            nc.vector.tensor_scalar(
                out=gvec[0:1, :], in0=gvec[0:1, :],
                scalar1=ab_sb[0:1, 0:1], scalar2=ab_sb[0:1, 1:2],
                op0=ALU.mult, op1=ALU.add)
            # broadcast to 128 partitions via rank-1 f32 matmul
            with tc.tile_pool(name="psg", bufs=1, space="PSUM") as ppg:
                gb_ps = ppg.tile([128, B_SH], f32)
                for bc in range(2):
                    nc.tensor.matmul(gb_ps[:, bc * 512:(bc + 1) * 512],
                                     ones_row[:],
                                     gvec[0:1, bc * 512:(bc + 1) * 512],
                                     start=True, stop=True)
                nc.scalar.activation(gb_sb[:], gb_ps[:], AFT.Copy)

            # second W2 half loads now that the x tiles' SBUF is free
            w2sp2_cm = tc.tile_pool(name="w2s2", bufs=1)
            w2sp2 = w2sp2_cm.__enter__()
            w2_sb[1] = w2sp2.tile([128, HT, 488], bf16, name="w2h1", tag="w2h1")
            for q4 in range(4):
                nc.sync.dma_start(
                    out=w2_sb[1][:, q4 * 8:(q4 + 1) * 8],
                    in_=W2p[:, q4 * 8:(q4 + 1) * 8, 512:1000])

            # ---- P2: h2 = g*h (in place), BN2 stats, chunked exchange ----
            def bn2_finish_chunk(q):
                ch = BN2_CHUNKS[q]
                f = int(bn2_first[q])
                nc.sync.dma_start(out=cc2_ins[q][:], in_=st2[q][:])
                nc.gpsimd.collective_compute(
                    "AllGather", mybir.AluOpType.bypass,
                    replica_groups=GROUPS,
                    ins=[cc2_ins[q][:]], outs=[cc2_outs[q][:]])
                nc.sync.dma_start(
                    out=ag2[q][:],
                    in_=cc2_outs[q].rearrange("r p c x -> p r c x"))
                ag_reduce(ag2[q], st2[q][:])
                bn_coeffs(st2[q][:, :, 0], st2[q][:, :, 1], scale2, bias2,
                          g2, bt2, f, ch, tmps[0], tmps[1], tmps[2])
                for i in range(ch):
                    ht = f + i
                    nc.scalar.activation(
                        z_sb[:, ht, :], z_sb[:, ht, :], AFT.Relu,
                        bias=bias2[:, ht:ht + 1], scale=scale2[:, ht:ht + 1])

            for ht in range(HT):
                q, i = bn2_chunk_of(ht)
                nc.vector.scalar_tensor_tensor(
                    out=z_sb[:, ht, :], in0=z_sb[:, ht, :], scalar=1.0,
                    in1=gb_sb[:], op0=ALU.mult, op1=ALU.mult,
                    accum_out=st2[q][:, i, 0:1])
                sq = sqp.tile([128, B_SH], bf16, tag="sq2")
                nc.scalar.activation(
                    sq[:], z_sb[:, ht, :], AFT.Square,
                    accum_out=st2[q][:, i, 1:2])
                if i == BN2_CHUNKS[q] - 1:
                    bn2_finish_chunk(q)

            # ---------------- P3: out = h3 @ W2 + b2 ----------------
            with tc.tile_pool(name="os", bufs=3) as osp, \
                 tc.tile_pool(name="ps3", bufs=1, space="PSUM") as pp3:
                for half, bts in enumerate((range(0, 4), range(4, 8))):
                    pss = {}
                    for oc, (o0, ow) in enumerate([(0, 512), (512, 488)]):
                        for bt in bts:
                            pss[(oc, bt)] = pp3.tile(
                                [128, 512], f32, tag=f"po{oc}_{bt % 4}",
                                name=f"po{oc}_{bt % 4}")
                    for ht in range(HT):
                        for oc, (o0, ow) in enumerate([(0, 512), (512, 488)]):
                            for bt in bts:
                                nc.tensor.matmul(
                                    pss[(oc, bt)][:, 0:ow],
                                    z_sb[:, ht, bt * 128:(bt + 1) * 128],
                                    w2_sb[oc][:, ht, :],
                                    start=(ht == 0), stop=(ht == HT - 1))
                    for oc, (o0, ow) in enumerate([(0, 512), (512, 488)]):
                        for bt in bts:
                            ot = osp.tile([128, 512], f32, tag="ot")
                            nc.vector.tensor_add(
                                ot[:, 0:ow], pss[(oc, bt)][:, 0:ow],
                                b2b[:, o0:o0 + ow])
                            nc.sync.dma_start(
                                out=out[bt * 128:(bt + 1) * 128, o0:o0 + ow],
                                in_=ot[:, 0:ow])

            w2sp2_cm.__exit__(None, None, None)
            w2sp_cm.__exit__(None, None, None)
            sqp_cm.__exit__(None, None, None)
            zp.__exit__(None, None, None)

    nc.compile()
    return nc


def _get_nc(mm_mode=None):
    nc = _BUILD_CACHE.get("nc")
    if nc is None:
        nc = _build()
        _BUILD_CACHE["nc"] = nc
    return nc


MM_MODE = "fp8dr"  # kept for test.py compatibility


def _quantize_terms_x(xT):
    """xT: [D_IN, B] f32 -> (xa, xb, xc) fp8 packed [128, KP, 2, B]."""
    def pack(a):
        return np.ascontiguousarray(
            a.reshape(KP, 2, 128, a.shape[1]).transpose(2, 0, 1, 3))
    xa = xT.astype(F8NP)
    rx = xT - xa.astype(np.float32)
    xb = (rx * S_XB).astype(F8NP)
    xc = (xT / 16.0).astype(F8NP)
    return pack(xa), pack(xb), pack(xc)


def _quantize_terms_w(W1):
    """W1: [D_IN, D_H] f32 -> [128, HG, 3, KP, 2, 256] fp8."""
    WS = W1.astype(np.float32) * np.float32(S_W)
    Wa = WS.astype(F8NP)
    rw = WS - Wa.astype(np.float32)
    Wb = (Wa.astype(np.float32) / S_XB).astype(F8NP)
    Wc = (rw * S_WC).astype(F8NP)
    terms = np.stack([np.asarray(Wa), np.asarray(Wb), np.asarray(Wc)], axis=0)
    # [3, D_IN, D_H] -> [3, KP, 2, 128, HG, 256] -> [128, HG, 3, KP, 2, 256]
    t = terms.reshape(3, KP, 2, 128, HG, 256).transpose(3, 4, 0, 1, 2, 5)
    return np.ascontiguousarray(t)


def kernel(**inputs):
    x = np.asarray(inputs["x"], np.float32)
    g_real = np.asarray(inputs["g_real"], np.float32)
    g_imag = np.asarray(inputs["g_imag"], np.float32)

    # Spectral filter must be (numerically) a delta for the fused fast path.
    ck = _filter_kernel(g_real, g_imag)
    delta = np.zeros_like(ck)
    delta[0] = 1.0
    ck_view = ck.view(np.float32) if ck.dtype == np.complex64 else ck.view(np.float64)
    if not (np.all(np.isfinite(ck_view)) and np.abs(ck - delta).max() < 1e-6):
        a = {k: np.asarray(v) for k, v in inputs.items()}
        return _np_reference(
            a["x"], a["W1"], a["b1"], a["gamma1"], a["beta1"], a["g_real"],
            a["g_imag"], float(a["alpha"][0]), float(a["beta_p"][0]),
            a["gamma2"], a["beta2"], a["W2"], a["b2"])

    from concourse.bass_utils import run_bass_kernel_spmd

    nc = _get_nc()

    def _pt(v):  # [4096] -> [128, 32] partition-major
        return np.ascontiguousarray(
            np.asarray(v, np.float32).reshape(HT, 128).T)

    W2 = np.asarray(inputs["W2"], np.float32)
    # (t p) o -> p t o, bf16
    W2p = np.ascontiguousarray(
        W2.reshape(HT, 128, D_OUT).transpose(1, 0, 2).astype(BF16NP))

    # scale by 8 keeps bf16 rounding behaviour identical; no scale needed
    shared = {
        "W1p": _quantize_terms_w(np.asarray(inputs["W1"], np.float32)),
        "gamma1": _pt(inputs["gamma1"]),
        "beta1": _pt(inputs["beta1"]),
        "gamma2": _pt(inputs["gamma2"]),
        "beta2": _pt(inputs["beta2"]),
        "alpha": np.ascontiguousarray(inputs["alpha"], dtype=np.float32),
        "beta_p": np.ascontiguousarray(inputs["beta_p"], dtype=np.float32),
        "W2p": W2p,
        "b2": np.ascontiguousarray(inputs["b2"], dtype=np.float32),
    }
    in_maps = []
    for c in range(N_CORES):
        sh = dict(shared)
        xT = np.ascontiguousarray(x[c * B_SH:(c + 1) * B_SH, :].T)
        xa, xb, xc = _quantize_terms_x(xT)
        sh["x_a"], sh["x_b"], sh["x_c"] = xa, xb, xc
        in_maps.append(sh)
    res = run_bass_kernel_spmd(nc, in_maps, list(range(N_CORES)))
    return np.concatenate(
        [res.results[c]["out"] for c in range(N_CORES)], axis=0)
